# revision 1
# baseline (speedup 1.0000x reference)
"""Trainium2 Bass kernel for nn_DigitConvolutionalModel (dense_cnn).

Math: the 3x3 valid conv is linear in x, so it folds into fc1:
    conv(x) @ fc1_w.T == x @ (C @ fc1_w.T)  with C [784, 676] the conv matrix.
The whole model is then a 3-layer MLP:
    out = relu(relu(x @ W1 + b1) @ W2 + b2) @ W3 + b3
with W1 = C @ fc1_w.T [784,512], W2 = fc2_w.T [512,512], W3 = out_w.T [512,10].

Sharding: pure data parallelism; batch 32768 -> 8 cores x 4096 rows.

On-chip formulation is fully transposed (features on SBUF partitions, batch on
the free dim): each layer computes h^T = act(W_l as lhsT, rhs = h_{l-1}^T).

PE-array scheduling (vs the 106us baseline):
 - Layer 1 contraction re-tiled as 6 full 128-row k-tiles + a 16-row
   remainder executed as ONE array pass of 4 concurrent row-tiled matmuls
   (tile_position=(32*mi,0), K=16), with x/w replicated across partition
   strips on the host. 28 -> ~25 array passes per chunk.
 - Layer 3 (M=10): the 4 k-tile matmuls are col-tiled (tile_position=
   (0,32*ki), M padded to 32 so the whole PSUM bank is written) into ONE
   concurrent pass; a DVE copy + one bf16 matmul against a 0/1 selection
   matrix reduces the 4 partition-strip partials.
 - The layer-3 tail of chunk n-2 (col-pack, DVE copy, reduce matmul) is
   injected between layer-1 k-groups of chunk n, so its ACT/DVE latencies
   never stall the in-order PE queue.
 - h2 relu+bias runs on the idle vector engine (scalar_tensor_tensor with a
   zeros tile) so the scalar engine only carries h1 relus + the final bias.
 - DMA: chunk-0 x / w1 single k-tiles interleaved across both HWDGE rings in
   PE first-use order; chunks 1-2 split; x1 before w2 (program order =
   layer1(1) runs before layer23(0)).
 - Warm-up: vector-engine memset + N=128 dummy matmuls from ~6us keep HAM at
   K=8/8 through the initial DMA fill; small bridges between chunk-0 k-groups.
"""

import numpy as np
import ml_dtypes

NCORES = 8
B = 32768
BC = B // NCORES  # rows per core
CH = 512          # batch chunk = matmul moving free dim = one fp32 PSUM bank
NCH = BC // CH
KF = 6            # layer-1 full k-tiles of 128 (remainder 16 rows row-packed)
MT1 = 4           # 512 out feats = 4 m-tiles of 128
KT2, MT2 = 4, 4   # layer-2: K=512, M=512
KT3, MO = 4, 10   # layer-3: K=512, M=10
XW = KF * CH + CH          # x chunk cols: 6 k-tiles + replicated k6 block
W1W = KF * 512 + 128       # w1 cols: 6 k-tiles + replicated k6 block
N_WARM = 16       # N=128 dummy matmuls bridging preamble -> first k-tile

_cache = {}


def _build():
    """Trace + compile the Bass program once per process."""
    if "nc" in _cache:
        return _cache["nc"]

    from contextlib import ExitStack

    import concourse.bass as bass
    import concourse.tile as tile
    from concourse import bacc, mybir
    from concourse.bass import ts, ds

    DT = mybir.dt.bfloat16
    F32 = mybir.dt.float32
    Relu = mybir.ActivationFunctionType.Relu
    Ident = mybir.ActivationFunctionType.Identity
    Add = mybir.AluOpType.add
    Max = mybir.AluOpType.max

    from concourse.vector_clock import ScopedClock

    class _FastExitTileContext(tile.TileContext):
        """Skip the exit semaphore-clear chain + second barrier (~2us tail)."""

        def _drain_and_barrier(self, tick_clock, wait_clock):
            drain_inst = self.nc.sync.drain()
            wait_clock.add_sem_waits(
                drain_inst.ins, ScopedClock({None: tick_clock.global_clock})
            )
            popped = self.nc._tile_sem_poison_stack.pop()
            assert popped is self._sem_poison

    nc = bacc.Bacc(
        "TRN2",
        target_bir_lowering=False,
        debug=False,
        enable_asserts=False,
        num_devices=NCORES,
        enable_partition_id=False,
    )

    xt_d = nc.dram_tensor("xt", [NCH, 128, XW], DT, kind="ExternalInput")
    w1_d = nc.dram_tensor("w1", [128, W1W], DT, kind="ExternalInput")
    w2_d = nc.dram_tensor("w2", [128, KT2 * 512], DT, kind="ExternalInput")
    # merged small consts, one DMA: w1 k6 block [128] | w3 k-tiles [4*10]
    cst_d = nc.dram_tensor("cst", [128, 168], DT, kind="ExternalInput")
    b_d = nc.dram_tensor("b", [128, MT1 + MT2 + 1], F32, kind="ExternalInput")
    out_d = nc.dram_tensor("out", [MO, BC], F32, kind="ExternalOutput")

    with _FastExitTileContext(nc) as tc, ExitStack() as ctx:
        consts = ctx.enter_context(tc.tile_pool(name="consts", bufs=1))
        xt_pool = ctx.enter_context(tc.tile_pool(name="xt", bufs=5))
        h1_pool = ctx.enter_context(tc.tile_pool(name="h1", bufs=8))
        h2_pool = ctx.enter_context(tc.tile_pool(name="h2", bufs=8))
        oc_pool = ctx.enter_context(tc.tile_pool(name="oc", bufs=2))
        ps1 = ctx.enter_context(tc.tile_pool(name="ps1", bufs=4, space="PSUM"))
        ps2 = ctx.enter_context(tc.tile_pool(name="ps2", bufs=3, space="PSUM"))
        ps3 = ctx.enter_context(tc.tile_pool(name="ps3", bufs=1, space="PSUM"))

        # --- PE pre-warm from the earliest post-preamble instant ---
        warm_sb = consts.tile([128, 128], DT, name="warm_sb")
        nc.vector.memset(warm_sb[:], 0.0)
        zeros = consts.tile([128, CH], DT, name="zeros")
        nc.vector.memset(zeros[:], 0.0)
        warm_ps = ps2.tile([128, 128], F32, name="warm_ps", tag="ps2")
        for _ in range(N_WARM):
            nc.tensor.matmul(warm_ps[:], warm_sb[:], warm_sb[:],
                             start=True, stop=True)

        def bridge(k):
            for _ in range(k):
                nc.tensor.matmul(warm_ps[:], warm_sb[:], warm_sb[:],
                                 start=True, stop=True)

        # --- input DMAs ---
        # Few, coarse DMAs: each DMA_DIRECT2D costs ~600ns of issuing-engine
        # time and too many in flight stall the engine on ring/semaphore
        # backpressure (which cascades into late activations). Chunk-0 x/w1
        # come as k-tile PAIRS interleaved across both rings in PE-deadline
        # order; late chunks are issued from inside the pipeline loop.
        # ring A = scalar engine queue, ring B = sync engine queue.
        b_sb = consts.tile([128, MT1 + MT2 + 1], F32, name="b_sb")
        nc.sync.dma_start(b_sb[:], b_d[:])

        # chunk-0: k0 and k1 as single tiles (earliest possible first real
        # matmul), then pairs
        x0k0 = consts.tile([128, CH], DT, name="x0k0")
        nc.scalar.dma_start(x0k0[:], xt_d[0][:, :CH])
        w1k0 = consts.tile([128, 512], DT, name="w1k0")
        nc.sync.dma_start(w1k0[:], w1_d[:, :512])
        w1k1 = consts.tile([128, 512], DT, name="w1k1")
        nc.scalar.dma_start(w1k1[:], w1_d[:, 512:1024])
        x0k1 = consts.tile([128, CH], DT, name="x0k1")
        nc.sync.dma_start(x0k1[:], xt_d[0][:, CH: 2 * CH])

        x0p = [None] * 3
        w1p = [None] * 3
        x0p[1] = consts.tile([128, 2 * CH], DT, name="x0p1")
        nc.scalar.dma_start(x0p[1][:], xt_d[0][:, 2 * CH: 4 * CH])
        w1p[1] = consts.tile([128, 1024], DT, name="w1p1")
        nc.sync.dma_start(w1p[1][:], w1_d[:, 1024:2048])

        x0p[2] = consts.tile([128, 2 * CH], DT, name="x0p2")
        nc.scalar.dma_start(x0p[2][:], xt_d[0][:, 4 * CH: 6 * CH])
        w1p[2] = consts.tile([128, 1024], DT, name="w1p2")
        nc.sync.dma_start(w1p[2][:], w1_d[:, 2048:3072])

        cst = consts.tile([128, 168], DT, name="cst")
        nc.sync.dma_start(cst[:], cst_d[:])
        x0k6 = consts.tile([128, CH], DT, name="x0k6")
        nc.sync.dma_start(x0k6[:], xt_d[0][:, KF * CH:])

        # chunk 1: k0-5 on the (faster) scalar ring, k6 block on sync;
        # needed before w2 (layer1(1) runs before layer2(0))
        x1a = consts.tile([128, KF * CH], DT, name="x1a")
        nc.scalar.dma_start(x1a[:], xt_d[1][:, : KF * CH])
        x1b = consts.tile([128, CH], DT, name="x1b")
        nc.sync.dma_start(x1b[:], xt_d[1][:, KF * CH:])

        w2_sb = consts.tile([128, KT2 * 512], DT, name="w2_sb")
        nc.scalar.dma_start(w2_sb[:], w2_d[:])

        x2a = consts.tile([128, 4 * CH], DT, name="x2a")
        nc.scalar.dma_start(x2a[:], xt_d[2][:, : 4 * CH])
        x2b = consts.tile([128, 3 * CH], DT, name="x2b")
        nc.sync.dma_start(x2b[:], xt_d[2][:, 4 * CH:])

        xtc = [None] * NCH

        def fetch(n):
            # late-chunk x DMAs issued from inside the pipeline (2 chunks of
            # lead) so at most a handful of DMAs are ever in flight
            t = xt_pool.tile([128, XW], DT, name=f"xtc{n}", tag="xtc")
            eng = nc.scalar if n % 2 == 1 else nc.sync
            eng.dma_start(t[:], xt_d[n])
            xtc[n] = t

        def w1s(ki, mi):
            if ki == 0:
                return w1k0[:, ds(mi * 128, 128)]
            if ki == 1:
                return w1k1[:, ds(mi * 128, 128)]
            return w1p[ki // 2][:, ds((ki % 2) * 512 + mi * 128, 128)]

        def xsl(n, ki):
            if n == 0:
                if ki == 0:
                    return x0k0[:]
                if ki == 1:
                    return x0k1[:]
                return x0p[ki // 2][:, ts(ki % 2, CH)]
            if n == 1:
                return x1a[:, ts(ki, CH)]
            if n == 2:
                return (x2a[:, ts(ki, CH)] if ki < 4
                        else x2b[:, ts(ki - 4, CH)])
            return xtc[n][:, ts(ki, CH)]

        def xk6(n, mi):
            if n == 0:
                t, o = x0k6, 0
            elif n == 1:
                t, o = x1b, 0
            elif n == 2:
                t, o = x2b, 2 * CH
            else:
                t, o = xtc[n], KF * CH
            return t[32 * mi:32 * mi + 16, ds(o, CH)]

        # ---- pipeline stages ----
        h1s = {}   # n -> h1 tiles
        h2s = {}   # n -> h2 tiles
        pscs = {}  # n -> layer-3 psum

        def l1_ki(n, ps, ki):
            for mi in range(MT1):
                nc.tensor.matmul(
                    ps[mi][:], w1s(ki, mi), xsl(n, ki),
                    start=(ki == 0), stop=False,
                )

        def l1_k6(n, ps):
            # 16-row remainder: 4 concurrent row-tiled matmuls, one pass
            for mi in range(MT1):
                nc.tensor.matmul(
                    ps[mi][:],
                    cst[32 * mi:32 * mi + 16, 0:128],
                    xk6(n, mi),
                    start=False, stop=True,
                    tile_position=(32 * mi, 0),
                )

        def l1_acts(n, ps):
            h1t = [
                h1_pool.tile([128, CH], DT, name=f"h1_{n}_{mi}", tag="h1")
                for mi in range(MT1)
            ]
            for mi in range(MT1):
                nc.scalar.activation(
                    h1t[mi][:], ps[mi][:], Relu, bias=b_sb[:, mi:mi + 1]
                )
            h1s[n] = h1t

        def l3_mms(n, kis):
            # layer-3 matmuls (M=10, accumulating): injected into a later
            # chunk's layer-1 stream, so all h2 inputs are long since ready
            if kis[0] == 0:
                # last chunk borrows a ps2 bank: ps3 has one buf, and in the
                # epilogue chunk 7 would otherwise wait for chunk 6's ACT
                pool = ps2 if n == NCH - 1 else ps3
                pscs[n] = pool.tile([MO, CH], F32, name=f"ps3_{n}",
                                    tag=pool is ps2 and "ps2" or "ps3")
            psc = pscs[n]
            h2t = h2s[n]
            for ki in kis:
                nc.tensor.matmul(
                    psc[:], cst[:, ds(128 + ki * MO, MO)], h2t[ki][:],
                    start=(ki == 0), stop=(ki == KT3 - 1),
                )

        def l3_fin(n):
            oct_ = oc_pool.tile([MO, CH], F32, name=f"oc_{n}", tag="oc")
            nc.scalar.activation(
                oct_[:], pscs[n][:], Ident,
                bias=b_sb[:MO, MT1 + MT2:MT1 + MT2 + 1],
            )
            nc.sync.dma_start(out_d[:, ts(n, CH)], oct_[:],
                              single_packet=True)

        def layer2(n):
            h1t = h1s.pop(n)
            h2t = [
                h2_pool.tile([128, CH], DT, name=f"h2_{n}_{mi}", tag="h2")
                for mi in range(MT2)
            ]
            for mi in range(MT2):
                ps = ps2.tile([128, CH], F32, name=f"ps2_{n}_{mi}", tag="ps2")
                for ki in range(KT2):
                    nc.tensor.matmul(
                        ps[:],
                        w2_sb[:, ds(ki * 512 + mi * 128, 128)],
                        h1t[ki][:],
                        start=(ki == 0), stop=(ki == KT2 - 1),
                    )
                # relu+bias on the vector engine: max(ps + b, 0)
                nc.vector.scalar_tensor_tensor(
                    h2t[mi][:], ps[:], b_sb[:, MT1 + mi:MT1 + mi + 1],
                    zeros[:], Add, Max,
                )
            h2s[n] = h2t

        # ---- software pipeline ----
        # iter n: layer1(n) with chunk n-2's layer-3 tail injected between
        # k-groups (all its inputs are ready -> zero PE stall), then layer2
        # of chunk n-1.
        for n in range(NCH):
            if 1 <= n <= NCH - 3:
                fetch(n + 2)
            ps = [
                ps1.tile([128, CH], F32, name=f"ps1_{n}_{mi}", tag="ps1")
                for mi in range(MT1)
            ]
            l1_ki(n, ps, 0)
            if n == 0:
                bridge(3)
            l1_ki(n, ps, 1)
            if n == 0:
                bridge(3)
            l1_ki(n, ps, 2)
            l1_ki(n, ps, 3)
            if n == 0:
                bridge(6)
            l1_ki(n, ps, 4)
            l1_ki(n, ps, 5)
            if n == 0:
                bridge(2)
            l1_k6(n, ps)
            # chunk n-2's layer 3 rides the small-shape transition the k6
            # pack already pays for
            if n >= 2:
                l3_mms(n - 2, (0, 1, 2, 3))
                l3_fin(n - 2)
            l1_acts(n, ps)
            if n >= 1:
                layer2(n - 1)

        # epilogue: drain chunks 6 and 7; chunk-7 relus overlap chunk-6 L3
        layer2(NCH - 1)
        l3_mms(NCH - 2, (0, 1, 2, 3))
        l3_fin(NCH - 2)
        l3_mms(NCH - 1, (0, 1, 2, 3))
        l3_fin(NCH - 1)

    nc.compile()
    _cache["nc"] = nc
    return nc


def _prep_inputs(x, conv_w, fc1_w, fc1_b, fc2_w, fc2_b, out_w, out_b):
    dt = ml_dtypes.bfloat16
    f32 = np.float32

    # Conv as a [784, 676] matrix (exact in fp64), folded into fc1.
    C = np.zeros((784, 676), dtype=np.float64)
    oy, ox = np.meshgrid(np.arange(26), np.arange(26), indexing="ij")
    cols = (oy * 26 + ox).ravel()
    for ky in range(3):
        for kx in range(3):
            rows = ((oy + ky) * 28 + (ox + kx)).ravel()
            np.add.at(C, (rows, cols), float(conv_w[ky, kx]))
    W1 = (C @ fc1_w.T.astype(np.float64)).astype(f32)  # [784, 512]

    # w1: 6 full k-tile blocks [128, 512] + replicated 16-row remainder
    w1 = np.zeros((128, W1W), dtype=f32)
    for ki in range(KF):
        w1[:, ki * 512:(ki + 1) * 512] = W1[ki * 128:(ki + 1) * 128, :]
    for j in range(4):
        w1[32 * j:32 * j + 16, KF * 512:] = W1[768:784, 128 * j:128 * (j + 1)]
    w1 = w1.astype(dt)

    w2 = np.ascontiguousarray(
        np.ascontiguousarray(fc2_w.T).reshape(KT2, 128, 512).transpose(1, 0, 2)
    ).reshape(128, KT2 * 512).astype(dt)
    b3col = np.zeros((128, 1), dtype=np.float64)
    b3col[:MO, 0] = out_b
    b = np.ascontiguousarray(
        np.concatenate(
            [fc1_b.reshape(MT1, 128).T, fc2_b.reshape(MT2, 128).T, b3col],
            axis=1,
        )
    ).astype(f32)
    # merged consts: w1 k6 block | w3 k-tiles
    cstm = np.zeros((128, 168), dtype=f32)
    cstm[:, 0:128] = w1[:, KF * 512:].astype(f32)
    cstm[:, 128:168] = np.ascontiguousarray(out_w.T).reshape(
        KT3, 128, MO).transpose(1, 0, 2).reshape(128, KT3 * MO)
    cstm = cstm.astype(dt)

    in_maps = []
    for c in range(NCORES):
        xc = x[c * BC:(c + 1) * BC].T.astype(dt, order="C")  # [784, BC]
        xch = np.zeros((NCH, 128, XW), dtype=dt)
        for n in range(NCH):
            cn = slice(n * CH, (n + 1) * CH)
            for ki in range(KF):
                xch[n, :, ki * CH:(ki + 1) * CH] = \
                    xc[ki * 128:(ki + 1) * 128, cn]
            rep = xc[768:784, cn]
            for j in range(4):
                xch[n, 32 * j:32 * j + 16, KF * CH:] = rep
        in_maps.append(
            {"xt": xch, "w1": w1, "w2": w2, "cst": cstm, "b": b}
        )
    return in_maps


def kernel(x, conv_w, fc1_w, fc1_b, fc2_w, fc2_b, out_w, out_b, _results=None):
    from concourse.bass_utils import run_bass_kernel_spmd

    x, conv_w, fc1_w, fc1_b, fc2_w, fc2_b, out_w, out_b = (
        np.asarray(a)
        for a in (x, conv_w, fc1_w, fc1_b, fc2_w, fc2_b, out_w, out_b)
    )
    nc = _build()
    in_maps = _prep_inputs(x, conv_w, fc1_w, fc1_b, fc2_w, fc2_b, out_w, out_b)
    res = run_bass_kernel_spmd(nc, in_maps, core_ids=list(range(NCORES)))
    if _results is not None:
        _results.append(res)
    out = np.empty((B, 10), dtype=np.float32)
    for c in range(NCORES):
        out[c * BC:(c + 1) * BC, :] = res.results[c]["out"].T
    return out



# revision 3
# speedup vs baseline: 1.1804x; 1.1804x over previous
"""Trainium2 Bass kernel for nn_DigitConvolutionalModel (dense_cnn).

Math: the 3x3 valid conv is linear in x, so it folds into fc1:
    conv(x) @ fc1_w.T == x @ (C @ fc1_w.T)  with C [784, 676] the conv matrix.
The whole model is then a 3-layer MLP:
    out = relu(relu(x @ W1 + b1) @ W2 + b2) @ W3 + b3
with W1 = C @ fc1_w.T [784,512], W2 = fc2_w.T [512,512], W3 = out_w.T [512,10].

Sharding: pure data parallelism; batch 32768 -> 8 cores x 4096 rows.

On-chip formulation is fully transposed (features on SBUF partitions, batch on
the free dim): each layer computes h^T = act(W_l as lhsT, rhs = h_{l-1}^T).

PE-array scheduling (HW-measured model):
 - flags!=3 (accumulating) matmuls drain PSUM at ~259ns/512 cols; flags=3
   (start+stop) at ~216ns. LDWEIGHTS is free (hidden by dual weight buffers).
 - Layer 1: 6 full 128-row k-tiles + a 16-row remainder executed as ONE array
   pass of 4 concurrent row-tiled matmuls (tile_position=(32*mi,0), K=16).
 - Layer 3 (M=10): ONE concurrent pass of 4 col-tiled flags=3 matmuls
   (tile_position=(0,32*ki)) writes the 4 k-partials to partition strips of
   one PSUM bank; a full-partition DVE copy (fp16) + one matmul against a 0/1
   selection matrix reduces the strips.  ~2 fast passes vs 4 slow ones.
 - The layer-3 stages of chunk n-2 are injected into chunk n's stream so
   their ACT/DVE latencies never stall the in-order PE queue.
 - h2 relu+bias runs on the idle vector engine (scalar_tensor_tensor with a
   zeros tile) so the scalar engine only carries h1 relus + the final bias.
 - DMA: chunk-0 x / w1 single k-tiles interleaved across both HWDGE rings in
   PE first-use order; chunks 1-2 split; x1 before w2 (program order =
   layer1(1) runs before layer23(0)).
 - Warm-up: vector-engine memset + N=128 dummy matmuls from ~4us keep HAM
   busy through the initial DMA fill so it reaches K=8/8 early; bridges
   between chunk-0 k-groups cover the DMA-wait gaps.
"""

import numpy as np
import ml_dtypes

NCORES = 8
B = 32768
BC = B // NCORES  # rows per core
CH = 512          # batch chunk = matmul moving free dim = one fp32 PSUM bank
NCH = BC // CH
KF = 6            # layer-1 full k-tiles of 128 (remainder 16 rows row-packed)
MT1 = 4           # 512 out feats = 4 m-tiles of 128
KT2, MT2 = 4, 4   # layer-2: K=512, M=512
KT3, MO = 4, 10   # layer-3: K=512, M=10
XW = KF * CH + CH          # x chunk cols: 6 k-tiles + replicated k6 block
W1W = KF * 512 + 128       # w1 cols: 6 k-tiles + replicated k6 block
N_WARM = 18       # N=128 dummy matmuls bridging preamble -> first k-tile

_cache = {}


def _build():
    """Trace + compile the Bass program once per process."""
    if "nc" in _cache:
        return _cache["nc"]

    from contextlib import ExitStack

    import concourse.bass as bass
    import concourse.tile as tile
    from concourse import bacc, mybir
    from concourse.bass import ts, ds

    DT = mybir.dt.bfloat16
    F16 = mybir.dt.float16
    F32 = mybir.dt.float32
    Relu = mybir.ActivationFunctionType.Relu
    Ident = mybir.ActivationFunctionType.Identity
    Add = mybir.AluOpType.add
    Max = mybir.AluOpType.max

    from concourse.vector_clock import ScopedClock

    class _FastExitTileContext(tile.TileContext):
        """Skip the exit semaphore-clear chain + second barrier (~2us tail)."""

        def _drain_and_barrier(self, tick_clock, wait_clock):
            drain_inst = self.nc.sync.drain()
            wait_clock.add_sem_waits(
                drain_inst.ins, ScopedClock({None: tick_clock.global_clock})
            )
            popped = self.nc._tile_sem_poison_stack.pop()
            assert popped is self._sem_poison

    nc = bacc.Bacc(
        "TRN2",
        target_bir_lowering=False,
        debug=False,
        enable_asserts=False,
        num_devices=NCORES,
        enable_partition_id=False,
    )

    xt_d = nc.dram_tensor("xt", [NCH, 128, XW], DT, kind="ExternalInput")
    w1_d = nc.dram_tensor("w1", [128, W1W], DT, kind="ExternalInput")
    w2_d = nc.dram_tensor("w2", [128, KT2 * 512], DT, kind="ExternalInput")
    # merged small consts, one DMA: w1 k6 block [128] | w3 k-tiles [4*10]
    cst_d = nc.dram_tensor("cst", [128, 168], DT, kind="ExternalInput")
    sel_d = nc.dram_tensor("sel", [128, 16], F16, kind="ExternalInput")
    b_d = nc.dram_tensor("b", [128, MT1 + MT2 + 1], F32, kind="ExternalInput")
    out_d = nc.dram_tensor("out", [MO, BC], F32, kind="ExternalOutput")

    with _FastExitTileContext(nc) as tc, ExitStack() as ctx:
        consts = ctx.enter_context(tc.tile_pool(name="consts", bufs=1))
        xt_pool = ctx.enter_context(tc.tile_pool(name="xt", bufs=5))
        h1_pool = ctx.enter_context(tc.tile_pool(name="h1", bufs=8))
        h2_pool = ctx.enter_context(tc.tile_pool(name="h2", bufs=8))
        sb3_pool = ctx.enter_context(tc.tile_pool(name="sb3", bufs=2))
        oc_pool = ctx.enter_context(tc.tile_pool(name="oc", bufs=2))
        ps1 = ctx.enter_context(tc.tile_pool(name="ps1", bufs=4, space="PSUM"))
        ps2 = ctx.enter_context(tc.tile_pool(name="ps2", bufs=3, space="PSUM"))
        ps3 = ctx.enter_context(tc.tile_pool(name="ps3", bufs=1, space="PSUM"))

        # --- PE pre-warm from the earliest post-preamble instant ---
        warm_sb = consts.tile([128, 128], DT, name="warm_sb")
        nc.vector.memset(warm_sb[:], 0.0)
        zeros = consts.tile([128, CH], DT, name="zeros")
        nc.vector.memset(zeros[:], 0.0)
        warm_ps = ps2.tile([128, 128], F32, name="warm_ps", tag="ps2")
        for _ in range(N_WARM):
            nc.tensor.matmul(warm_ps[:], warm_sb[:], warm_sb[:],
                             start=True, stop=True)

        def bridge(k):
            for _ in range(k):
                nc.tensor.matmul(warm_ps[:], warm_sb[:], warm_sb[:],
                                 start=True, stop=True)

        # --- input DMAs ---
        # Few, coarse DMAs: each DMA_DIRECT2D costs ~600ns of issuing-engine
        # time and too many in flight stall the engine on ring/semaphore
        # backpressure (which cascades into late activations). Chunk-0 x/w1
        # come as k-tile PAIRS interleaved across both rings in PE-deadline
        # order; late chunks are issued from inside the pipeline loop.
        # ring A = scalar engine queue, ring B = sync engine queue.
        b_sb = consts.tile([128, MT1 + MT2 + 1], F32, name="b_sb")
        nc.sync.dma_start(b_sb[:], b_d[:])

        # chunk-0: k0 and k1 as single tiles (earliest possible first real
        # matmul), then pairs
        x0k0 = consts.tile([128, CH], DT, name="x0k0")
        nc.scalar.dma_start(x0k0[:], xt_d[0][:, :CH])
        w1k0 = consts.tile([128, 512], DT, name="w1k0")
        nc.sync.dma_start(w1k0[:], w1_d[:, :512])
        w1k1 = consts.tile([128, 512], DT, name="w1k1")
        nc.scalar.dma_start(w1k1[:], w1_d[:, 512:1024])
        x0k1 = consts.tile([128, CH], DT, name="x0k1")
        nc.sync.dma_start(x0k1[:], xt_d[0][:, CH: 2 * CH])

        x0p = [None] * 3
        w1p = [None] * 3
        x0p[1] = consts.tile([128, 2 * CH], DT, name="x0p1")
        nc.scalar.dma_start(x0p[1][:], xt_d[0][:, 2 * CH: 4 * CH])
        w1p[1] = consts.tile([128, 1024], DT, name="w1p1")
        nc.sync.dma_start(w1p[1][:], w1_d[:, 1024:2048])

        x0p[2] = consts.tile([128, 2 * CH], DT, name="x0p2")
        nc.scalar.dma_start(x0p[2][:], xt_d[0][:, 4 * CH: 6 * CH])
        w1p[2] = consts.tile([128, 1024], DT, name="w1p2")
        nc.sync.dma_start(w1p[2][:], w1_d[:, 2048:3072])

        cst = consts.tile([128, 168], DT, name="cst")
        nc.sync.dma_start(cst[:], cst_d[:])
        sel_sb = consts.tile([128, 16], F16, name="sel_sb")
        nc.sync.dma_start(sel_sb[:], sel_d[:])
        x0k6 = consts.tile([128, CH], DT, name="x0k6")
        nc.sync.dma_start(x0k6[:], xt_d[0][:, KF * CH:])

        # chunk 1: k0-5 on the (faster) scalar ring, k6 block on sync;
        # needed before w2 (layer1(1) runs before layer2(0))
        x1a = consts.tile([128, KF * CH], DT, name="x1a")
        nc.scalar.dma_start(x1a[:], xt_d[1][:, : KF * CH])
        x1b = consts.tile([128, CH], DT, name="x1b")
        nc.sync.dma_start(x1b[:], xt_d[1][:, KF * CH:])

        w2_sb = consts.tile([128, KT2 * 512], DT, name="w2_sb")
        nc.scalar.dma_start(w2_sb[:], w2_d[:])

        x2a = consts.tile([128, 4 * CH], DT, name="x2a")
        nc.scalar.dma_start(x2a[:], xt_d[2][:, : 4 * CH])
        x2b = consts.tile([128, 3 * CH], DT, name="x2b")
        nc.sync.dma_start(x2b[:], xt_d[2][:, 4 * CH:])

        xtc = [None] * NCH

        def fetch(n):
            # late-chunk x DMAs issued from inside the pipeline (2 chunks of
            # lead) so at most a handful of DMAs are ever in flight
            t = xt_pool.tile([128, XW], DT, name=f"xtc{n}", tag="xtc")
            eng = nc.scalar if n % 2 == 1 else nc.sync
            eng.dma_start(t[:], xt_d[n])
            xtc[n] = t

        def w1s(ki, mi):
            if ki == 0:
                return w1k0[:, ds(mi * 128, 128)]
            if ki == 1:
                return w1k1[:, ds(mi * 128, 128)]
            return w1p[ki // 2][:, ds((ki % 2) * 512 + mi * 128, 128)]

        def xsl(n, ki):
            if n == 0:
                if ki == 0:
                    return x0k0[:]
                if ki == 1:
                    return x0k1[:]
                return x0p[ki // 2][:, ts(ki % 2, CH)]
            if n == 1:
                return x1a[:, ts(ki, CH)]
            if n == 2:
                return (x2a[:, ts(ki, CH)] if ki < 4
                        else x2b[:, ts(ki - 4, CH)])
            return xtc[n][:, ts(ki, CH)]

        def xk6(n, mi):
            if n == 0:
                t, o = x0k6, 0
            elif n == 1:
                t, o = x1b, 0
            elif n == 2:
                t, o = x2b, 2 * CH
            else:
                t, o = xtc[n], KF * CH
            return t[32 * mi:32 * mi + 16, ds(o, CH)]

        # ---- pipeline stages ----
        h1s = {}   # n -> h1 tiles
        h2s = {}   # n -> h2 tiles

        # single persistent layer-3 PSUM bank; strips at partitions 32k..+10.
        # memset once: stale NaN/Inf in the never-written rows would poison
        # the 0-weighted lanes of the selection reduce (0*NaN = NaN).
        ps3_t = ps3.tile([128, CH], F32, name="ps3_t", tag="ps3")
        nc.vector.memset(ps3_t[:], 0.0)
        sb3s = {}  # n -> fp16 strip-copy tile

        def l1_ki(n, ps, ki):
            for mi in range(MT1):
                nc.tensor.matmul(
                    ps[mi][:], w1s(ki, mi), xsl(n, ki),
                    start=(ki == 0), stop=False,
                )

        def l1_k6(n, ps):
            # 16-row remainder: 4 concurrent row-tiled matmuls, one pass
            for mi in range(MT1):
                nc.tensor.matmul(
                    ps[mi][:],
                    cst[32 * mi:32 * mi + 16, 0:128],
                    xk6(n, mi),
                    start=False, stop=True,
                    tile_position=(32 * mi, 0),
                )

        def l1_acts(n, ps):
            h1t = [
                h1_pool.tile([128, CH], DT, name=f"h1_{n}_{mi}", tag="h1")
                for mi in range(MT1)
            ]
            for mi in range(MT1):
                nc.scalar.activation(
                    h1t[mi][:], ps[mi][:], Relu, bias=b_sb[:, mi:mi + 1]
                )
            h1s[n] = h1t

        def l3_pack(n):
            # ONE concurrent pass: 4 col-tiled flags=3 matmuls write the 4
            # k-partials of chunk n to partition strips {32k..32k+10} of the
            # shared bank, then a full-partition fp16 DVE copy grabs them.
            h2t = h2s.pop(n)
            for ki in range(KT3):
                nc.tensor.matmul(
                    ps3_t[32 * ki:32 * ki + MO, :],
                    cst[:, ds(128 + ki * MO, MO)], h2t[ki][:],
                    start=True, stop=True,
                    tile_position=(0, 32 * ki),
                )
            sb = sb3_pool.tile([128, CH], F16, name=f"sb3_{n}", tag="sb3")
            nc.vector.tensor_copy(sb[:], ps3_t[:])
            sb3s[n] = sb

        def l3_fin(n):
            # reduce the 4 strips with a 0/1 selection matmul (contracts
            # over partitions), then bias + store
            nc.tensor.matmul(ps3_t[:MO, :], sel_sb[:, :MO], sb3s.pop(n)[:],
                             start=True, stop=True)
            oct_ = oc_pool.tile([MO, CH], F32, name=f"oc_{n}", tag="oc")
            nc.scalar.activation(
                oct_[:], ps3_t[:MO, :], Ident,
                bias=b_sb[:MO, MT1 + MT2:MT1 + MT2 + 1],
            )
            nc.sync.dma_start(out_d[:, ts(n, CH)], oct_[:],
                              single_packet=True)

        def layer2(n, mis):
            if mis[0] == 0:
                h2s[n] = [None] * MT2
            h1t = h1s[n]
            h2t = h2s[n]
            for mi in mis:
                ps = ps2.tile([128, CH], F32, name=f"ps2_{n}_{mi}", tag="ps2")
                for ki in range(KT2):
                    nc.tensor.matmul(
                        ps[:],
                        w2_sb[:, ds(ki * 512 + mi * 128, 128)],
                        h1t[ki][:],
                        start=(ki == 0), stop=(ki == KT2 - 1),
                    )
                h2t[mi] = h2_pool.tile([128, CH], DT, name=f"h2_{n}_{mi}",
                                       tag="h2")
                # relu+bias on the vector engine: max(ps + b, 0)
                nc.vector.scalar_tensor_tensor(
                    h2t[mi][:], ps[:], b_sb[:, MT1 + mi:MT1 + mi + 1],
                    zeros[:], Add, Max,
                )
            if mis[-1] == MT2 - 1:
                h1s.pop(n)

        # ---- software pipeline ----
        # iter n: layer1(n) with chunk n-2's layer-3 pack injected after the
        # k6 pass (all its inputs long ready -> zero PE stall); the layer-3
        # reduce+fin ride between layer2(n-1) mi-groups so the DVE copy
        # latency is hidden.
        for n in range(NCH):
            if 1 <= n <= NCH - 3:
                fetch(n + 2)
            ps = [
                ps1.tile([128, CH], F32, name=f"ps1_{n}_{mi}", tag="ps1")
                for mi in range(MT1)
            ]
            l1_ki(n, ps, 0)
            if n == 0:
                bridge(4)
            l1_ki(n, ps, 1)
            if n == 0:
                bridge(4)
            l1_ki(n, ps, 2)
            l1_ki(n, ps, 3)
            if n == 0:
                bridge(7)
            l1_ki(n, ps, 4)
            l1_ki(n, ps, 5)
            if n == 0:
                bridge(3)
            l1_k6(n, ps)
            if n >= 2:
                l3_pack(n - 2)
            l1_acts(n, ps)
            if n >= 1:
                layer2(n - 1, (0, 1))
                if n >= 2:
                    l3_fin(n - 2)
                layer2(n - 1, (2, 3))

        # epilogue: drain chunks 6 and 7; chunk-6 L3 rides between chunk-7
        # L2 mi-groups, chunk-7 L3 is the unavoidable tail chain
        layer2(NCH - 1, (0, 1))
        l3_pack(NCH - 2)
        layer2(NCH - 1, (2, 3))
        l3_fin(NCH - 2)
        l3_pack(NCH - 1)
        l3_fin(NCH - 1)

    nc.compile()
    _cache["nc"] = nc
    return nc


def _prep_inputs(x, conv_w, fc1_w, fc1_b, fc2_w, fc2_b, out_w, out_b):
    dt = ml_dtypes.bfloat16
    f32 = np.float32

    # Conv as a [784, 676] matrix (exact in fp64), folded into fc1.
    C = np.zeros((784, 676), dtype=np.float64)
    oy, ox = np.meshgrid(np.arange(26), np.arange(26), indexing="ij")
    cols = (oy * 26 + ox).ravel()
    for ky in range(3):
        for kx in range(3):
            rows = ((oy + ky) * 28 + (ox + kx)).ravel()
            np.add.at(C, (rows, cols), float(conv_w[ky, kx]))
    W1 = (C @ fc1_w.T.astype(np.float64)).astype(f32)  # [784, 512]

    # w1: 6 full k-tile blocks [128, 512] + replicated 16-row remainder
    w1 = np.zeros((128, W1W), dtype=f32)
    for ki in range(KF):
        w1[:, ki * 512:(ki + 1) * 512] = W1[ki * 128:(ki + 1) * 128, :]
    for j in range(4):
        w1[32 * j:32 * j + 16, KF * 512:] = W1[768:784, 128 * j:128 * (j + 1)]
    w1 = w1.astype(dt)

    w2 = np.ascontiguousarray(
        np.ascontiguousarray(fc2_w.T).reshape(KT2, 128, 512).transpose(1, 0, 2)
    ).reshape(128, KT2 * 512).astype(dt)
    b3col = np.zeros((128, 1), dtype=np.float64)
    b3col[:MO, 0] = out_b
    b = np.ascontiguousarray(
        np.concatenate(
            [fc1_b.reshape(MT1, 128).T, fc2_b.reshape(MT2, 128).T, b3col],
            axis=1,
        )
    ).astype(f32)
    # merged consts: w1 k6 block | w3 k-tiles
    cstm = np.zeros((128, 168), dtype=f32)
    cstm[:, 0:128] = w1[:, KF * 512:].astype(f32)
    cstm[:, 128:168] = np.ascontiguousarray(out_w.T).reshape(
        KT3, 128, MO).transpose(1, 0, 2).reshape(128, KT3 * MO)
    cstm = cstm.astype(dt)
    # 0/1 selection matrix: out row j sums partition strips {32q+j}
    sel = np.zeros((128, 16), dtype=np.float16)
    for j in range(MO):
        for q in range(KT3):
            sel[32 * q + j, j] = 1.0

    in_maps = []
    for c in range(NCORES):
        xc = x[c * BC:(c + 1) * BC].T.astype(dt, order="C")  # [784, BC]
        xch = np.zeros((NCH, 128, XW), dtype=dt)
        for n in range(NCH):
            cn = slice(n * CH, (n + 1) * CH)
            for ki in range(KF):
                xch[n, :, ki * CH:(ki + 1) * CH] = \
                    xc[ki * 128:(ki + 1) * 128, cn]
            rep = xc[768:784, cn]
            for j in range(4):
                xch[n, 32 * j:32 * j + 16, KF * CH:] = rep
        in_maps.append(
            {"xt": xch, "w1": w1, "w2": w2, "cst": cstm, "sel": sel, "b": b}
        )
    return in_maps


def kernel(x, conv_w, fc1_w, fc1_b, fc2_w, fc2_b, out_w, out_b, _results=None):
    from concourse.bass_utils import run_bass_kernel_spmd

    x, conv_w, fc1_w, fc1_b, fc2_w, fc2_b, out_w, out_b = (
        np.asarray(a)
        for a in (x, conv_w, fc1_w, fc1_b, fc2_w, fc2_b, out_w, out_b)
    )
    nc = _build()
    in_maps = _prep_inputs(x, conv_w, fc1_w, fc1_b, fc2_w, fc2_b, out_w, out_b)
    res = run_bass_kernel_spmd(nc, in_maps, core_ids=list(range(NCORES)))
    if _results is not None:
        _results.append(res)
    out = np.empty((B, 10), dtype=np.float32)
    for c in range(NCORES):
        out[c * BC:(c + 1) * BC, :] = res.results[c]["out"].T
    return out
